# revision 11
# baseline (speedup 1.0000x reference)
"""MinibatchDiscrimination kernel for 8 Trainium2 NeuronCores.

reference:
    m = einsum('bi,iok->bok', x, T)          # B=128, IN=1024, OUT=512, K=16
    norm[i,j,o] = sum_k |m[j,o,k] - m[i,o,k]|
    o_b = sum_i exp(-norm) - 1               # [B, OUT]
    out = concat([x, o_b], axis=1)           # [128, 1536]

Sharding: each core owns OUT/8 = 64 output features (zero communication).

Per-core pipeline (pair-matmul, strictly-upper-triangular):
  1. GEMM on PE: m[b, f] = x @ T_c, f = o_local*16 + k (F = 1024, 8 f-tiles).
  2. Pair differences on PE: for f-tile t, diff[f, pair] = m_t.T @ psel where
     psel[b, (i,j)] = +1{b==i} - 1{b==j} over the 8128 pairs i<j. Streamed in
     [128, 512] PSUM chunks.
  3. |diff| -> SBUF bf16: ACT tiles use one Abs op per chunk; DVE tiles use
     two fused ops (relu(d), relu(-d)) into separate planes (the add is
     folded into the k-reduce contraction width).
  4. k-reduce + i-stacking on PE: per i one matmul over its pair block,
     selector S32_a [128, 32] with tile_position=(0, 32q) packs 16 i's into
     one [128, 128] group (row = 32*(isub//4) + 8*(isub%4) + osub); four
     groups share one PSUM bank [128, 512]; matmul start=True zeroes the
     bank once, so unwritten (j <= i) columns are exact zeros.
  5. exp(-norm) on ACT over [128, 512]; zeros exp to exactly 1.0 -> the
     deterministic junk is removed host-side (po[o,j] -= 128-j, rowsum -= i+1).
  6. Column sums: selector matmul S2_t [128, 64] accumulates over everything
     into PSUM [64, 128]. Row sums: DVE tensor_reduce -> [128, 64] table.
  7. Host: o_b[j, o] = (po[o, j] - (128-j)) + reindexed rowsums.
i==j pairs are never computed, so no "-1" correction is needed.
"""

import numpy as np
import ml_dtypes

import concourse.bass as bass
import concourse.tile as tile
from concourse import mybir
from concourse.bass_utils import run_bass_kernel_spmd

BF16 = mybir.dt.bfloat16
F32 = mybir.dt.float32
A = mybir.AluOpType
AF = mybir.ActivationFunctionType

B = 128
IN = 1024
OUT = 512
K = 16
NCORES = 8
OC = OUT // NCORES       # 64
F = OC * K               # 1024
NT = F // 128            # 8 f-tiles
NCI = IN // 128          # 8 contraction chunks
NPAIR = (B * (B - 1)) // 2   # 8128 strictly-upper pairs
CHUNK = 512
NCHUNK = (NPAIR + CHUNK - 1) // CHUNK   # 16 (last = 448)

# which f-tiles run their |diff| on DVE (two relu planes) vs ACT (one Abs op)
DVE_TILES = (False, False, False, True, False, False, False, True)


def _pair_base(i):
    return i * 127 - (i * (i - 1)) // 2


def _split_excess_waits(nc, max_waits=1):
    """This walrus build rejects instructions carrying more than one sem
    wait; hoist extras onto preceding NoOps on the same engine."""
    for fn in nc.m.functions:
        for blk in fn.blocks:
            new_insts = []
            for inst in blk.instructions:
                si = inst.sync_info
                if si and si.on_wait and len(si.on_wait) > max_waits:
                    waits = list(si.on_wait)
                    extra, keep = waits[:-max_waits], waits[-max_waits:]
                    k = 0
                    while extra:
                        chunk, extra = extra[:max_waits], extra[max_waits:]
                        nop = mybir.InstNoOp(
                            name=f"{inst.name}-ws{k}", engine=inst.engine,
                            ins=[], outs=[],
                            sync_info=mybir.SyncInfo(on_wait=chunk, on_update=[]))
                        nc.register_instruction(nop)
                        new_insts.append(nop)
                        k += 1
                    inst.sync_info = mybir.SyncInfo(
                        on_wait=keep, on_update=list(si.on_update))
                new_insts.append(inst)
            blk.instructions[:] = new_insts


def _emit_pd_abs(nc, pools, t, m_bf, psel_sb):
    """Pair-diff matmuls + |.| for one f-tile; returns the absdiff tile."""
    work, ework, pdiff, pnorm = pools
    dve = DVE_TILES[t]
    planes = 2 if dve else 1
    absd = work.tile([128, planes, NPAIR], BF16, tag="absd")
    for c in range(NCHUNK):
        lo = c * CHUNK
        w = min(CHUNK, NPAIR - lo)
        pd = pdiff.tile([128, CHUNK], F32, tag="pd")
        nc.tensor.matmul(pd[:, 0:w], m_bf[:, 128 * t:128 * (t + 1)],
                         psel_sb[:, lo:lo + w], start=True, stop=True)
        if dve:
            nc.vector.tensor_scalar(absd[:, 0, lo:lo + w], pd[:, 0:w],
                                    0.0, None, op0=A.max)
            nc.vector.tensor_scalar(absd[:, 1, lo:lo + w], pd[:, 0:w],
                                    -1.0, 0.0, op0=A.mult, op1=A.max)
        else:
            nc.scalar.activation(absd[:, 0, lo:lo + w], pd[:, 0:w], AF.Abs)
    return absd


def _emit_kred(nc, pools, t, absd, s32_sb, s2_sb, po, rs_all):
    """k-reduce (packed, strip-interleaved) + exp + row/col sums for tile t."""
    work, ework, pdiff, pnorm = pools
    dve = DVE_TILES[t]
    for G in range(2):
        pn = pnorm.tile([128, 512], F32, tag="pn")
        # zero the full tile: cells no matmul writes (j <= i) must read as
        # exact 0 so exp gives exactly 1.0 (host subtracts the known count)
        nc.vector.memset(pn[:], 0.0)
        first = True
        for gl in range(4):
            ig = 4 * G + gl
            for idx in range(16):
                # strip-interleaved: consecutive matmuls hit different
                # 32-col PE strips (q fastest) so they run concurrently
                q, a = idx % 4, idx // 4
                i = 16 * ig + 4 * a + q
                if i >= B - 1:
                    continue
                w = 127 - i
                bs = _pair_base(i)
                out_ap = pn[32 * q:32 * q + 32,
                            128 * gl + i + 1:128 * (gl + 1)]
                last = (gl == 3 and idx == 15)
                nc.tensor.matmul(
                    out_ap, s32_sb[a][:], absd[:, 0, bs:bs + w],
                    start=first, stop=(last and not dve),
                    tile_position=(0, 32 * q), skip_group_check=True)
                first = False
                if dve:
                    # second relu plane accumulates into the same columns
                    nc.tensor.matmul(
                        out_ap, s32_sb[a][:], absd[:, 1, bs:bs + w],
                        start=False, stop=last,
                        tile_position=(0, 32 * q), skip_group_check=True)
        e = ework.tile([128, 512], BF16, tag="e")
        nc.scalar.activation(e[:], pn[:], AF.Exp, scale=-1.0)
        # row sums over j within each igroup -> rs_all[:, 8*ig + t]
        rs_view = rs_all.rearrange("p (ig tt) -> p ig tt", tt=8)
        nc.vector.tensor_reduce(
            rs_view[:, 4 * G:4 * G + 4, t],
            e[:].rearrange("p (g j) -> p g j", g=4), op=A.add,
            axis=mybir.AxisListType.X)
        for gl in range(4):
            ig = 4 * G + gl
            nc.tensor.matmul(po[:], s2_sb[t][:],
                             e[:, 128 * gl:128 * (gl + 1)],
                             start=(t == 0 and ig == 0),
                             stop=(t == NT - 1 and ig == 7))


def _build_program():
    nc = bass.Bass()
    xT_d = nc.dram_tensor("xt", [IN, B], BF16, kind="ExternalInput")
    tc_d = nc.dram_tensor("tc", [IN, F], BF16, kind="ExternalInput")
    psel_d = nc.dram_tensor("psel", [B, NPAIR], BF16, kind="ExternalInput")
    s32_d = nc.dram_tensor("s32", [4, 128, 32], BF16, kind="ExternalInput")
    s2_d = nc.dram_tensor("s2", [NT, 128, OC], BF16, kind="ExternalInput")
    po_d = nc.dram_tensor("po", [OC, B], F32, kind="ExternalOutput")
    rs_d = nc.dram_tensor("rs", [128, 64], F32, kind="ExternalOutput")

    with tile.TileContext(nc) as tc:
        with (
            tc.tile_pool(name="cst", bufs=1) as cst,
            tc.tile_pool(name="work", bufs=3) as work,
            tc.tile_pool(name="ework", bufs=4) as ework,
            tc.tile_pool(name="pgemm", bufs=1, space="PSUM") as pgemm,
            tc.tile_pool(name="pdiff", bufs=4, space="PSUM") as pdiff,
            tc.tile_pool(name="pnorm", bufs=2, space="PSUM") as pnorm,
            tc.tile_pool(name="pob", bufs=1, space="PSUM") as pob,
        ):
            xT_sb, tc_sb = [], []
            for ci in range(NCI):
                t_ = cst.tile([128, F], BF16, tag=f"tc{ci}")
                nc.sync.dma_start(t_[:], tc_d[128 * ci:128 * (ci + 1), :])
                tc_sb.append(t_)
                x_ = cst.tile([128, B], BF16, tag=f"xt{ci}")
                nc.sync.dma_start(x_[:], xT_d[128 * ci:128 * (ci + 1), :])
                xT_sb.append(x_)
            psel_sb = cst.tile([128, NPAIR], BF16, tag="psel")
            nc.sync.dma_start(psel_sb[:], psel_d[:])
            s32_sb = []
            for a in range(4):
                t_ = cst.tile([128, 32], BF16, tag=f"s32_{a}")
                nc.sync.dma_start(t_[:], s32_d[a])
                s32_sb.append(t_)
            s2_sb = []
            for t in range(NT):
                t_ = cst.tile([128, OC], BF16, tag=f"s2{t}")
                nc.sync.dma_start(t_[:], s2_d[t])
                s2_sb.append(t_)

            # ---- GEMM: m[b, f] = x @ T_c ----
            m_bf = cst.tile([128, F], BF16, tag="mbf")
            for half in range(2):
                ps = pgemm.tile([128, 512], F32, tag="pg")
                for ci in range(NCI):
                    nc.tensor.matmul(
                        ps[:], xT_sb[ci][:],
                        tc_sb[ci][:, 512 * half:512 * (half + 1)],
                        start=(ci == 0), stop=(ci == NCI - 1))
                nc.scalar.activation(m_bf[:, 512 * half:512 * (half + 1)],
                                     ps[:], AF.Copy, scale=1.0)

            po = pob.tile([OC, B], F32, tag="po")
            rs_all = cst.tile([128, 64], F32, tag="rs")

            # staggered emission: pair-diff/abs for tile t+1 goes into the
            # PE queue before k-reduce of tile t, so the PE never idles
            # behind the abs pass of the tile it is about to reduce
            pools = (work, ework, pdiff, pnorm)
            absd_tiles = {}
            absd_tiles[0] = _emit_pd_abs(nc, pools, 0, m_bf, psel_sb)
            for t in range(NT):
                if t + 1 < NT:
                    absd_tiles[t + 1] = _emit_pd_abs(nc, pools, t + 1,
                                                     m_bf, psel_sb)
                _emit_kred(nc, pools, t, absd_tiles.pop(t),
                           s32_sb, s2_sb, po, rs_all)

            po_sb = cst.tile([OC, B], F32, tag="posb")
            nc.vector.tensor_copy(po_sb[:], po[:])
            nc.sync.dma_start(po_d[:], po_sb[:])
            nc.sync.dma_start(rs_d[:], rs_all[:])

    _split_excess_waits(nc)
    return nc


def _host_consts():
    psel = np.zeros((B, NPAIR), np.float32)
    col = 0
    for i in range(B - 1):
        w = 127 - i
        psel[i, col:col + w] = 1.0
        psel[np.arange(i + 1, 128), np.arange(col, col + w)] = -1.0
        col += w
    s32 = np.zeros((4, 128, 32), np.float32)
    for a in range(4):
        for osub in range(8):
            s32[a, 16 * osub:16 * (osub + 1), 8 * a + osub] = 1.0
    s2 = np.zeros((NT, 128, OC), np.float32)
    for t in range(NT):
        for p in range(128):
            s2[t, p, 8 * t + (p % 8)] = 1.0
    return (psel.astype(ml_dtypes.bfloat16), s32.astype(ml_dtypes.bfloat16),
            s2.astype(ml_dtypes.bfloat16))


_CACHE = {}


def _get_cached():
    if "nc" not in _CACHE:
        _CACHE["nc"] = _build_program()
        _CACHE["consts"] = _host_consts()
        # rowsum reindex: rs_all[p, 8*ig + t] belongs to
        # i = 16*ig + 4*a + q with p = 32*q + 8*a + osub, o = 8*t + osub
        p_idx = np.arange(128)
        q, rem = p_idx // 32, p_idx % 32
        a_, osub = rem // 8, rem % 8
        cols = np.arange(64)
        ig, t_ = cols // 8, cols % 8
        i_map = 16 * ig[None, :] + 4 * a_[:, None] + q[:, None]   # [128, 64]
        o_map = 8 * t_[None, :] + osub[:, None]                   # [128, 64]
        _CACHE["i_map"] = i_map
        _CACHE["o_map"] = o_map
    return _CACHE


def kernel(x: np.ndarray, T: np.ndarray, _trace=False, _tmpdir=None) -> np.ndarray:
    x = np.asarray(x, dtype=np.float32)
    T = np.asarray(T, dtype=np.float32)
    c = _get_cached()
    nc = c["nc"]
    psel, s32, s2 = c["consts"]

    xt = np.ascontiguousarray(x.T).astype(ml_dtypes.bfloat16)
    in_maps = []
    for cr in range(NCORES):
        tc_c = np.ascontiguousarray(
            T[:, OC * cr:OC * (cr + 1), :].reshape(IN, F)
        ).astype(ml_dtypes.bfloat16)
        in_maps.append({"xt": xt, "tc": tc_c, "psel": psel,
                        "s32": s32, "s2": s2})

    kw = {}
    if _trace:
        kw = dict(trace=True, tmpdir=_tmpdir)
    res = run_bass_kernel_spmd(nc, in_maps, list(range(NCORES)), **kw)

    jj = np.arange(B, dtype=np.float32)
    junk_col = (B - jj)[None, :]          # po[o, j] junk = 128 - j
    i_map, o_map = c["i_map"], c["o_map"]
    o_b = np.empty((B, OUT), np.float32)
    for cr in range(NCORES):
        r = res.results[cr]
        po = r["po"] - junk_col                       # [64, 128] colsums
        ob_c = po.T.copy()                            # [j, o_local]
        rows = r["rs"] - (i_map + 1)                  # rowsums minus junk
        np.add.at(ob_c, (i_map.ravel(), o_map.ravel()), rows.ravel())
        o_b[:, OC * cr:OC * (cr + 1)] = ob_c
    out = np.concatenate([x, o_b], axis=1)
    if _trace:
        return out, res
    return out


# revision 15
# speedup vs baseline: 1.2213x; 1.2213x over previous
"""MinibatchDiscrimination kernel for 8 Trainium2 NeuronCores.

reference:
    m = einsum('bi,iok->bok', x, T)          # B=128, IN=1024, OUT=512, K=16
    norm[i,j,o] = sum_k |m[j,o,k] - m[i,o,k]|
    o_b = sum_i exp(-norm) - 1               # [B, OUT]
    out = concat([x, o_b], axis=1)           # [128, 1536]

Sharding: each core owns OUT/8 = 64 output features (zero communication).

Per-core pipeline (pair-matmul, strictly-upper-triangular):
  1. GEMM on PE: m[b, f] = x @ T_c, f = o_local*16 + k (F = 1024, 8 f-tiles).
  2. Pair differences on PE: for f-tile t, diff[f, pair] = m_t.T @ psel where
     psel[b, (i,j)] = +1{b==i} - 1{b==j} over the 8128 pairs i<j. Streamed in
     [128, 512] PSUM chunks.
  3. |diff| -> SBUF bf16: ACT tiles use one Abs op per chunk; DVE tiles use
     two fused ops (relu(d), relu(-d)) into separate planes (the add is
     folded into the k-reduce contraction width).
  4. k-reduce + i-stacking on PE: per i one matmul over its pair block,
     selector S32_a [128, 32] with tile_position=(0, 32q) packs 16 i's into
     one [128, 128] group (row = 32*(isub//4) + 8*(isub%4) + osub); four
     groups share one PSUM bank [128, 512]; matmul start=True zeroes the
     bank once, so unwritten (j <= i) columns are exact zeros.
  5. exp(-norm) on ACT over [128, 512]; zeros exp to exactly 1.0 -> the
     deterministic junk is removed host-side (po[o,j] -= 128-j, rowsum -= i+1).
  6. Column sums: selector matmul S2_t [128, 64] accumulates over everything
     into PSUM [64, 128]. Row sums: DVE tensor_reduce -> [128, 64] table.
  7. Host: o_b[j, o] = (po[o, j] - (128-j)) + reindexed rowsums.
i==j pairs are never computed, so no "-1" correction is needed.
"""

import numpy as np
import ml_dtypes

import concourse.bass as bass
import concourse.tile as tile
from concourse import mybir
from concourse.bass_utils import run_bass_kernel_spmd

BF16 = mybir.dt.bfloat16
F32 = mybir.dt.float32
A = mybir.AluOpType
AF = mybir.ActivationFunctionType

B = 128
IN = 1024
OUT = 512
K = 16
NCORES = 8
OC = OUT // NCORES       # 64
F = OC * K               # 1024
NT = F // 128            # 8 f-tiles
NCI = IN // 128          # 8 contraction chunks
NPAIR = (B * (B - 1)) // 2   # 8128 strictly-upper pairs
CHUNK = 512
NCHUNK = (NPAIR + CHUNK - 1) // CHUNK   # 16 (last = 448)

# which f-tiles run their |diff| on DVE (two relu planes) vs ACT (one Abs op)
DVE_TILES = (False, False, False, True, False, False, False, True)


def _pair_base(i):
    return i * 127 - (i * (i - 1)) // 2


def _split_excess_waits(nc, max_waits=1):
    """This walrus build rejects instructions carrying more than one sem
    wait; hoist extras onto preceding NoOps on the same engine."""
    for fn in nc.m.functions:
        for blk in fn.blocks:
            new_insts = []
            for inst in blk.instructions:
                si = inst.sync_info
                if si and si.on_wait and len(si.on_wait) > max_waits:
                    waits = list(si.on_wait)
                    extra, keep = waits[:-max_waits], waits[-max_waits:]
                    k = 0
                    while extra:
                        chunk, extra = extra[:max_waits], extra[max_waits:]
                        nop = mybir.InstNoOp(
                            name=f"{inst.name}-ws{k}", engine=inst.engine,
                            ins=[], outs=[],
                            sync_info=mybir.SyncInfo(on_wait=chunk, on_update=[]))
                        nc.register_instruction(nop)
                        new_insts.append(nop)
                        k += 1
                    inst.sync_info = mybir.SyncInfo(
                        on_wait=keep, on_update=list(si.on_update))
                new_insts.append(inst)
            blk.instructions[:] = new_insts


def _make_pd_abs_steps(nc, pools, t, m_bf, psel_sb):
    """Returns (absd_tile, steps): each step emits one pair-diff chunk
    matmul + its |.| op(s) when called."""
    work, ework, pdiff, pnorm = pools
    dve = DVE_TILES[t]
    planes = 2 if dve else 1
    absd = work.tile([128, planes, NPAIR], BF16, tag="absd")

    def step(c):
        lo = c * CHUNK
        w = min(CHUNK, NPAIR - lo)
        pd = pdiff.tile([128, CHUNK], F32, tag="pd")
        nc.tensor.matmul(pd[:, 0:w], m_bf[:, 128 * t:128 * (t + 1)],
                         psel_sb[:, lo:lo + w], start=True, stop=True)
        if dve:
            nc.vector.tensor_scalar(absd[:, 0, lo:lo + w], pd[:, 0:w],
                                    0.0, None, op0=A.max)
            nc.vector.tensor_scalar(absd[:, 1, lo:lo + w], pd[:, 0:w],
                                    -1.0, 0.0, op0=A.mult, op1=A.max)
        else:
            nc.scalar.activation(absd[:, 0, lo:lo + w], pd[:, 0:w], AF.Abs)

    return absd, [lambda c=c: step(c) for c in range(NCHUNK)]


def _emit_kred(nc, pools, t, absd, s32_sb, s2_sb, po, rs_all, weave=None):
    """k-reduce (packed, strip-interleaved) + exp + row/col sums for tile t.
    `weave` is a list of pending pair-diff steps for the NEXT tile; they are
    interleaved into the PE stream so the next tile's abs pass (ACT/DVE)
    overlaps this tile's k-reduce (PE)."""
    work, ework, pdiff, pnorm = pools
    dve = DVE_TILES[t]
    weave = list(weave or [])
    n_mm = 8 * 16 * (2 if dve else 1)
    stride = max(1, n_mm // (len(weave) + 1)) if weave else 0
    mm_count = 0

    def tick():
        nonlocal mm_count
        mm_count += 1
        if weave and stride and mm_count % stride == 0:
            weave.pop(0)()
    for G in range(2):
        pn = pnorm.tile([128, 512], F32, tag="pn")
        # zero the full tile: cells no matmul writes (j <= i) must read as
        # exact 0 so exp gives exactly 1.0 (host subtracts the known count)
        nc.vector.memset(pn[:], 0.0)
        first = True
        for gl in range(4):
            ig = 4 * G + gl
            for idx in range(16):
                # strip-interleaved: consecutive matmuls hit different
                # 32-col PE strips (q fastest) so they run concurrently
                q, a = idx % 4, idx // 4
                i = 16 * ig + 4 * a + q
                if i >= B - 1:
                    continue
                w = 127 - i
                bs = _pair_base(i)
                out_ap = pn[32 * q:32 * q + 32,
                            128 * gl + i + 1:128 * (gl + 1)]
                last = (gl == 3 and idx == 15)
                nc.tensor.matmul(
                    out_ap, s32_sb[a][:], absd[:, 0, bs:bs + w],
                    start=first, stop=(last and not dve),
                    tile_position=(0, 32 * q), skip_group_check=True)
                first = False
                tick()
                if dve:
                    # second relu plane accumulates into the same columns
                    nc.tensor.matmul(
                        out_ap, s32_sb[a][:], absd[:, 1, bs:bs + w],
                        start=False, stop=last,
                        tile_position=(0, 32 * q), skip_group_check=True)
                    tick()
        e = ework.tile([128, 512], BF16, tag="e")
        nc.scalar.activation(e[:], pn[:], AF.Exp, scale=-1.0)
        # row sums over j within each igroup -> rs_all[:, 8*ig + t]
        rs_view = rs_all.rearrange("p (ig tt) -> p ig tt", tt=8)
        nc.vector.tensor_reduce(
            rs_view[:, 4 * G:4 * G + 4, t],
            e[:].rearrange("p (g j) -> p g j", g=4), op=A.add,
            axis=mybir.AxisListType.X)
        for gl in range(4):
            ig = 4 * G + gl
            nc.tensor.matmul(po[:], s2_sb[t][:],
                             e[:, 128 * gl:128 * (gl + 1)],
                             start=(t == 0 and ig == 0),
                             stop=(t == NT - 1 and ig == 7))
    # flush any unwoven pair-diff steps for the next tile
    for stp in weave:
        stp()


def _build_program():
    nc = bass.Bass()
    xT_d = nc.dram_tensor("xt", [IN, B], BF16, kind="ExternalInput")
    tc_d = nc.dram_tensor("tc", [IN, F], BF16, kind="ExternalInput")
    psel_d = nc.dram_tensor("psel", [B, NPAIR], BF16, kind="ExternalInput")
    s32_d = nc.dram_tensor("s32", [4, 128, 32], BF16, kind="ExternalInput")
    s2_d = nc.dram_tensor("s2", [NT, 128, OC], BF16, kind="ExternalInput")
    po_d = nc.dram_tensor("po", [OC, B], F32, kind="ExternalOutput")
    rs_d = nc.dram_tensor("rs", [128, 64], F32, kind="ExternalOutput")

    with tile.TileContext(nc) as tc:
        with (
            tc.tile_pool(name="cst", bufs=1) as cst,
            tc.tile_pool(name="work", bufs=3) as work,
            tc.tile_pool(name="ework", bufs=4) as ework,
            tc.tile_pool(name="pgemm", bufs=1, space="PSUM") as pgemm,
            tc.tile_pool(name="pdiff", bufs=4, space="PSUM") as pdiff,
            tc.tile_pool(name="pnorm", bufs=2, space="PSUM") as pnorm,
            tc.tile_pool(name="pob", bufs=1, space="PSUM") as pob,
        ):
            xT_sb, tc_sb = [], []
            for ci in range(NCI):
                t_ = cst.tile([128, F], BF16, tag=f"tc{ci}")
                nc.sync.dma_start(t_[:], tc_d[128 * ci:128 * (ci + 1), :])
                tc_sb.append(t_)
                x_ = cst.tile([128, B], BF16, tag=f"xt{ci}")
                nc.sync.dma_start(x_[:], xT_d[128 * ci:128 * (ci + 1), :])
                xT_sb.append(x_)
            psel_sb = cst.tile([128, NPAIR], BF16, tag="psel")
            nc.sync.dma_start(psel_sb[:], psel_d[:])
            s32_sb = []
            for a in range(4):
                t_ = cst.tile([128, 32], BF16, tag=f"s32_{a}")
                nc.sync.dma_start(t_[:], s32_d[a])
                s32_sb.append(t_)
            s2_sb = []
            for t in range(NT):
                t_ = cst.tile([128, OC], BF16, tag=f"s2{t}")
                nc.sync.dma_start(t_[:], s2_d[t])
                s2_sb.append(t_)

            # ---- GEMM: m[b, f] = x @ T_c ----
            m_bf = cst.tile([128, F], BF16, tag="mbf")
            for half in range(2):
                ps = pgemm.tile([128, 512], F32, tag="pg")
                for ci in range(NCI):
                    nc.tensor.matmul(
                        ps[:], xT_sb[ci][:],
                        tc_sb[ci][:, 512 * half:512 * (half + 1)],
                        start=(ci == 0), stop=(ci == NCI - 1))
                nc.scalar.activation(m_bf[:, 512 * half:512 * (half + 1)],
                                     ps[:], AF.Copy, scale=1.0)

            po = pob.tile([OC, B], F32, tag="po")
            rs_all = cst.tile([128, 64], F32, tag="rs")

            # software pipeline: tile t's k-reduce (PE) interleaves the
            # pair-diff chunks of tile t+1, so t+1's abs pass (ACT/DVE)
            # overlaps t's k-reduce instead of serializing after it
            pools = (work, ework, pdiff, pnorm)
            absd0, steps0 = _make_pd_abs_steps(nc, pools, 0, m_bf, psel_sb)
            for s in steps0:
                s()
            cur_absd = absd0
            for t in range(NT):
                if t + 1 < NT:
                    nxt_absd, nxt_steps = _make_pd_abs_steps(
                        nc, pools, t + 1, m_bf, psel_sb)
                else:
                    nxt_absd, nxt_steps = None, []
                _emit_kred(nc, pools, t, cur_absd, s32_sb, s2_sb,
                           po, rs_all, weave=nxt_steps)
                cur_absd = nxt_absd

            po_sb = cst.tile([OC, B], F32, tag="posb")
            nc.vector.tensor_copy(po_sb[:], po[:])
            nc.sync.dma_start(po_d[:], po_sb[:])
            nc.sync.dma_start(rs_d[:], rs_all[:])

    _split_excess_waits(nc)
    return nc


def _host_consts():
    psel = np.zeros((B, NPAIR), np.float32)
    col = 0
    for i in range(B - 1):
        w = 127 - i
        psel[i, col:col + w] = 1.0
        psel[np.arange(i + 1, 128), np.arange(col, col + w)] = -1.0
        col += w
    s32 = np.zeros((4, 128, 32), np.float32)
    for a in range(4):
        for osub in range(8):
            s32[a, 16 * osub:16 * (osub + 1), 8 * a + osub] = 1.0
    s2 = np.zeros((NT, 128, OC), np.float32)
    for t in range(NT):
        for p in range(128):
            s2[t, p, 8 * t + (p % 8)] = 1.0
    return (psel.astype(ml_dtypes.bfloat16), s32.astype(ml_dtypes.bfloat16),
            s2.astype(ml_dtypes.bfloat16))


_CACHE = {}


def _get_cached():
    if "nc" not in _CACHE:
        _CACHE["nc"] = _build_program()
        _CACHE["consts"] = _host_consts()
        # rowsum reindex: rs_all[p, 8*ig + t] belongs to
        # i = 16*ig + 4*a + q with p = 32*q + 8*a + osub, o = 8*t + osub
        p_idx = np.arange(128)
        q, rem = p_idx // 32, p_idx % 32
        a_, osub = rem // 8, rem % 8
        cols = np.arange(64)
        ig, t_ = cols // 8, cols % 8
        i_map = 16 * ig[None, :] + 4 * a_[:, None] + q[:, None]   # [128, 64]
        o_map = 8 * t_[None, :] + osub[:, None]                   # [128, 64]
        _CACHE["i_map"] = i_map
        _CACHE["o_map"] = o_map
    return _CACHE


def kernel(x: np.ndarray, T: np.ndarray, _trace=False, _tmpdir=None) -> np.ndarray:
    x = np.asarray(x, dtype=np.float32)
    T = np.asarray(T, dtype=np.float32)
    c = _get_cached()
    nc = c["nc"]
    psel, s32, s2 = c["consts"]

    xt = np.ascontiguousarray(x.T).astype(ml_dtypes.bfloat16)
    in_maps = []
    for cr in range(NCORES):
        tc_c = np.ascontiguousarray(
            T[:, OC * cr:OC * (cr + 1), :].reshape(IN, F)
        ).astype(ml_dtypes.bfloat16)
        in_maps.append({"xt": xt, "tc": tc_c, "psel": psel,
                        "s32": s32, "s2": s2})

    kw = {}
    if _trace:
        kw = dict(trace=True, tmpdir=_tmpdir)
    res = run_bass_kernel_spmd(nc, in_maps, list(range(NCORES)), **kw)

    jj = np.arange(B, dtype=np.float32)
    junk_col = (B - jj)[None, :]          # po[o, j] junk = 128 - j
    i_map, o_map = c["i_map"], c["o_map"]
    o_b = np.empty((B, OUT), np.float32)
    for cr in range(NCORES):
        r = res.results[cr]
        po = r["po"] - junk_col                       # [64, 128] colsums
        ob_c = po.T.copy()                            # [j, o_local]
        rows = r["rs"] - (i_map + 1)                  # rowsums minus junk
        np.add.at(ob_c, (i_map.ravel(), o_map.ravel()), rows.ravel())
        o_b[:, OC * cr:OC * (cr + 1)] = ob_c
    out = np.concatenate([x, o_b], axis=1)
    if _trace:
        return out, res
    return out


# revision 18
# speedup vs baseline: 1.2586x; 1.0306x over previous
"""MinibatchDiscrimination kernel for 8 Trainium2 NeuronCores.

reference:
    m = einsum('bi,iok->bok', x, T)          # B=128, IN=1024, OUT=512, K=16
    norm[i,j,o] = sum_k |m[j,o,k] - m[i,o,k]|
    o_b = sum_i exp(-norm) - 1               # [B, OUT]
    out = concat([x, o_b], axis=1)           # [128, 1536]

Sharding: each core owns OUT/8 = 64 output features (zero communication).

Per-core pipeline (pair-matmul, strictly-upper-triangular):
  1. GEMM on PE: m[b, f] = x @ T_c, f = o_local*16 + k (F = 1024, 8 f-tiles).
  2. Pair differences on PE: for f-tile t, diff[f, pair] = m_t.T @ psel where
     psel[b, (i,j)] = +1{b==i} - 1{b==j} over the 8128 pairs i<j. Streamed in
     [128, 512] PSUM chunks.
  3. |diff| -> SBUF bf16: ACT tiles use one Abs op per chunk; DVE tiles use
     two fused ops (relu(d), relu(-d)) into separate planes (the add is
     folded into the k-reduce contraction width).
  4. k-reduce + i-stacking on PE: per i one matmul over its pair block,
     selector S32_a [128, 32] with tile_position=(0, 32q) packs 16 i's into
     one [128, 128] group (row = 32*(isub//4) + 8*(isub%4) + osub); four
     groups share one PSUM bank [128, 512]; matmul start=True zeroes the
     bank once, so unwritten (j <= i) columns are exact zeros.
  5. exp(-norm) on ACT over [128, 512]; zeros exp to exactly 1.0 -> the
     deterministic junk is removed host-side (po[o,j] -= 128-j, rowsum -= i+1).
  6. Column sums: selector matmul S2_t [128, 64] accumulates over everything
     into PSUM [64, 128]. Row sums: DVE tensor_reduce -> [128, 64] table.
  7. Host: o_b[j, o] = (po[o, j] - (128-j)) + reindexed rowsums.
i==j pairs are never computed, so no "-1" correction is needed.
"""

import numpy as np
import ml_dtypes

import concourse.bass as bass
import concourse.tile as tile
from concourse import mybir
from concourse.bass_utils import run_bass_kernel_spmd

BF16 = mybir.dt.bfloat16
F32 = mybir.dt.float32
A = mybir.AluOpType
AF = mybir.ActivationFunctionType

B = 128
IN = 1024
OUT = 512
K = 16
NCORES = 8
OC = OUT // NCORES       # 64
F = OC * K               # 1024
NT = F // 128            # 8 f-tiles
NCI = IN // 128          # 8 contraction chunks
NPAIR = (B * (B - 1)) // 2   # 8128 strictly-upper pairs
CHUNK = 512
NCHUNK = (NPAIR + CHUNK - 1) // CHUNK   # 16 (last = 448)

# which f-tiles run their |diff| on DVE (two relu planes) vs ACT (one Abs op)
DVE_TILES = (False, False, False, True, False, False, False, True)


def _pair_base(i):
    return i * 127 - (i * (i - 1)) // 2


def _split_excess_waits(nc, max_waits=1):
    """This walrus build rejects instructions carrying more than one sem
    wait; hoist extras onto preceding NoOps on the same engine."""
    for fn in nc.m.functions:
        for blk in fn.blocks:
            new_insts = []
            for inst in blk.instructions:
                si = inst.sync_info
                if si and si.on_wait and len(si.on_wait) > max_waits:
                    waits = list(si.on_wait)
                    extra, keep = waits[:-max_waits], waits[-max_waits:]
                    k = 0
                    while extra:
                        chunk, extra = extra[:max_waits], extra[max_waits:]
                        nop = mybir.InstNoOp(
                            name=f"{inst.name}-ws{k}", engine=inst.engine,
                            ins=[], outs=[],
                            sync_info=mybir.SyncInfo(on_wait=chunk, on_update=[]))
                        nc.register_instruction(nop)
                        new_insts.append(nop)
                        k += 1
                    inst.sync_info = mybir.SyncInfo(
                        on_wait=keep, on_update=list(si.on_update))
                new_insts.append(inst)
            blk.instructions[:] = new_insts


def _make_pd_abs_steps(nc, pools, t, m_bf, psel_sb):
    """Returns (absd_tile, steps): each step emits one pair-diff chunk
    matmul + its |.| op(s) when called."""
    work, ework, pdiff, pnorm = pools
    dve = DVE_TILES[t]
    planes = 2 if dve else 1
    absd = work.tile([128, planes, NPAIR], BF16, tag="absd")

    def step(c):
        lo = c * CHUNK
        w = min(CHUNK, NPAIR - lo)
        pd = pdiff.tile([128, CHUNK], F32, tag="pd")
        nc.tensor.matmul(pd[:, 0:w], m_bf[:, 128 * t:128 * (t + 1)],
                         psel_sb[:, lo:lo + w], start=True, stop=True)
        if dve:
            nc.vector.tensor_scalar(absd[:, 0, lo:lo + w], pd[:, 0:w],
                                    0.0, None, op0=A.max)
            nc.vector.tensor_scalar(absd[:, 1, lo:lo + w], pd[:, 0:w],
                                    -1.0, 0.0, op0=A.mult, op1=A.max)
        else:
            nc.scalar.activation(absd[:, 0, lo:lo + w], pd[:, 0:w], AF.Abs)

    return absd, [lambda c=c: step(c) for c in range(NCHUNK)]


def _emit_kred(nc, pools, t, absd, s32_sb, s2_sb, po, rs_all, weave=None):
    """k-reduce (packed, strip-interleaved) + exp + row/col sums for tile t.
    `weave` is a list of pending pair-diff steps for the NEXT tile; they are
    interleaved into the PE stream so the next tile's abs pass (ACT/DVE)
    overlaps this tile's k-reduce (PE)."""
    work, ework, pdiff, pnorm = pools
    dve = DVE_TILES[t]
    weave = list(weave or [])
    n_mm = 8 * 16 * (2 if dve else 1)
    stride = max(1, n_mm // (len(weave) + 1)) if weave else 0
    mm_count = 0

    def tick():
        nonlocal mm_count
        mm_count += 1
        if weave and stride and mm_count % stride == 0:
            weave.pop(0)()
    for G in range(2):
        pn = pnorm.tile([128, 512], F32, tag="pn")
        # zero the full tile: cells no matmul writes (j <= i) must read as
        # exact 0 so exp gives exactly 1.0 (host subtracts the known count)
        nc.vector.memset(pn[:], 0.0)
        first = True
        for gl in range(4):
            ig = 4 * G + gl
            for idx in range(16):
                # strip-interleaved: consecutive matmuls hit different
                # 32-col PE strips (q fastest) so they run concurrently
                q, a = idx % 4, idx // 4
                i = 16 * ig + 4 * a + q
                if i >= B - 1:
                    continue
                w = 127 - i
                bs = _pair_base(i)
                out_ap = pn[32 * q:32 * q + 32,
                            128 * gl + i + 1:128 * (gl + 1)]
                last = (gl == 3 and idx == 15)
                nc.tensor.matmul(
                    out_ap, s32_sb[a][:], absd[:, 0, bs:bs + w],
                    start=first, stop=(last and not dve),
                    tile_position=(0, 32 * q), skip_group_check=True)
                first = False
                tick()
                if dve:
                    # second relu plane accumulates into the same columns
                    nc.tensor.matmul(
                        out_ap, s32_sb[a][:], absd[:, 1, bs:bs + w],
                        start=False, stop=last,
                        tile_position=(0, 32 * q), skip_group_check=True)
                    tick()
        e = ework.tile([128, 512], BF16, tag="e")
        nc.scalar.activation(e[:], pn[:], AF.Exp, scale=-1.0)
        # row sums over j within each igroup -> rs_all[:, 8*ig + t]
        rs_view = rs_all.rearrange("p (ig tt) -> p ig tt", tt=8)
        nc.vector.tensor_reduce(
            rs_view[:, 4 * G:4 * G + 4, t],
            e[:].rearrange("p (g j) -> p g j", g=4), op=A.add,
            axis=mybir.AxisListType.X)
        for gl in range(4):
            ig = 4 * G + gl
            nc.tensor.matmul(po[:], s2_sb[t][:],
                             e[:, 128 * gl:128 * (gl + 1)],
                             start=(t == 0 and ig == 0),
                             stop=(t == NT - 1 and ig == 7))
    # flush any unwoven pair-diff steps for the next tile
    for stp in weave:
        stp()


def _build_program():
    nc = bass.Bass()
    xT_d = nc.dram_tensor("xt", [IN, B], BF16, kind="ExternalInput")
    tc_d = nc.dram_tensor("tc", [IN, F], BF16, kind="ExternalInput")
    psel_d = nc.dram_tensor("psel", [B, NPAIR], BF16, kind="ExternalInput")
    s32_d = nc.dram_tensor("s32", [4, 128, 32], BF16, kind="ExternalInput")
    s2_d = nc.dram_tensor("s2", [NT, 128, OC], BF16, kind="ExternalInput")
    po_d = nc.dram_tensor("po", [OC, B], F32, kind="ExternalOutput")
    rs_d = nc.dram_tensor("rs", [128, 64], F32, kind="ExternalOutput")

    with tile.TileContext(nc) as tc:
        with (
            tc.tile_pool(name="cst", bufs=1) as cst,
            tc.tile_pool(name="work", bufs=3) as work,
            tc.tile_pool(name="ework", bufs=4) as ework,
            tc.tile_pool(name="pgemm", bufs=1, space="PSUM") as pgemm,
            tc.tile_pool(name="pdiff", bufs=4, space="PSUM") as pdiff,
            tc.tile_pool(name="pnorm", bufs=2, space="PSUM") as pnorm,
            tc.tile_pool(name="pob", bufs=1, space="PSUM") as pob,
        ):
            xT_sb, tc_sb = [], []
            for ci in range(NCI):
                t_ = cst.tile([128, F], BF16, tag=f"tc{ci}")
                nc.sync.dma_start(t_[:], tc_d[128 * ci:128 * (ci + 1), :])
                tc_sb.append(t_)
                x_ = cst.tile([128, B], BF16, tag=f"xt{ci}")
                nc.sync.dma_start(x_[:], xT_d[128 * ci:128 * (ci + 1), :])
                xT_sb.append(x_)
            # per-chunk DMA so the first pair-diff matmul can start as soon
            # as its slice (and m_bf) lands, not after the full 2MB
            psel_sb = cst.tile([128, NPAIR], BF16, tag="psel")
            for cch in range(NCHUNK):
                lo = cch * CHUNK
                w = min(CHUNK, NPAIR - lo)
                nc.sync.dma_start(psel_sb[:, lo:lo + w], psel_d[:, lo:lo + w])
            s32_sb = []
            for a in range(4):
                t_ = cst.tile([128, 32], BF16, tag=f"s32_{a}")
                nc.sync.dma_start(t_[:], s32_d[a])
                s32_sb.append(t_)
            s2_sb = []
            for t in range(NT):
                t_ = cst.tile([128, OC], BF16, tag=f"s2{t}")
                nc.sync.dma_start(t_[:], s2_d[t])
                s2_sb.append(t_)

            # ---- GEMM: m[b, f] = x @ T_c ----
            m_bf = cst.tile([128, F], BF16, tag="mbf")
            for half in range(2):
                ps = pgemm.tile([128, 512], F32, tag="pg")
                for ci in range(NCI):
                    nc.tensor.matmul(
                        ps[:], xT_sb[ci][:],
                        tc_sb[ci][:, 512 * half:512 * (half + 1)],
                        start=(ci == 0), stop=(ci == NCI - 1))
                nc.scalar.activation(m_bf[:, 512 * half:512 * (half + 1)],
                                     ps[:], AF.Copy, scale=1.0)

            po = pob.tile([OC, B], F32, tag="po")
            rs_all = cst.tile([128, 64], F32, tag="rs")

            # software pipeline: tile t's k-reduce (PE) interleaves the
            # pair-diff chunks of tile t+1, so t+1's abs pass (ACT/DVE)
            # overlaps t's k-reduce instead of serializing after it
            pools = (work, ework, pdiff, pnorm)
            absd0, steps0 = _make_pd_abs_steps(nc, pools, 0, m_bf, psel_sb)
            for s in steps0:
                s()
            cur_absd = absd0
            for t in range(NT):
                if t + 1 < NT:
                    nxt_absd, nxt_steps = _make_pd_abs_steps(
                        nc, pools, t + 1, m_bf, psel_sb)
                else:
                    nxt_absd, nxt_steps = None, []
                _emit_kred(nc, pools, t, cur_absd, s32_sb, s2_sb,
                           po, rs_all, weave=nxt_steps)
                cur_absd = nxt_absd

            po_sb = cst.tile([OC, B], F32, tag="posb")
            nc.vector.tensor_copy(po_sb[:], po[:])
            nc.sync.dma_start(po_d[:], po_sb[:])
            nc.sync.dma_start(rs_d[:], rs_all[:])

    _split_excess_waits(nc)
    return nc


def _host_consts():
    psel = np.zeros((B, NPAIR), np.float32)
    col = 0
    for i in range(B - 1):
        w = 127 - i
        psel[i, col:col + w] = 1.0
        psel[np.arange(i + 1, 128), np.arange(col, col + w)] = -1.0
        col += w
    s32 = np.zeros((4, 128, 32), np.float32)
    for a in range(4):
        for osub in range(8):
            s32[a, 16 * osub:16 * (osub + 1), 8 * a + osub] = 1.0
    s2 = np.zeros((NT, 128, OC), np.float32)
    for t in range(NT):
        for p in range(128):
            s2[t, p, 8 * t + (p % 8)] = 1.0
    return (psel.astype(ml_dtypes.bfloat16), s32.astype(ml_dtypes.bfloat16),
            s2.astype(ml_dtypes.bfloat16))


_CACHE = {}


def _get_cached():
    if "nc" not in _CACHE:
        _CACHE["nc"] = _build_program()
        _CACHE["consts"] = _host_consts()
        # rowsum reindex: rs_all[p, 8*ig + t] belongs to
        # i = 16*ig + 4*a + q with p = 32*q + 8*a + osub, o = 8*t + osub
        p_idx = np.arange(128)
        q, rem = p_idx // 32, p_idx % 32
        a_, osub = rem // 8, rem % 8
        cols = np.arange(64)
        ig, t_ = cols // 8, cols % 8
        i_map = 16 * ig[None, :] + 4 * a_[:, None] + q[:, None]   # [128, 64]
        o_map = 8 * t_[None, :] + osub[:, None]                   # [128, 64]
        _CACHE["i_map"] = i_map
        _CACHE["o_map"] = o_map
    return _CACHE


def kernel(x: np.ndarray, T: np.ndarray, _trace=False, _tmpdir=None) -> np.ndarray:
    x = np.asarray(x, dtype=np.float32)
    T = np.asarray(T, dtype=np.float32)
    c = _get_cached()
    nc = c["nc"]
    psel, s32, s2 = c["consts"]

    xt = np.ascontiguousarray(x.T).astype(ml_dtypes.bfloat16)
    in_maps = []
    for cr in range(NCORES):
        tc_c = np.ascontiguousarray(
            T[:, OC * cr:OC * (cr + 1), :].reshape(IN, F)
        ).astype(ml_dtypes.bfloat16)
        in_maps.append({"xt": xt, "tc": tc_c, "psel": psel,
                        "s32": s32, "s2": s2})

    kw = {}
    if _trace:
        kw = dict(trace=True, tmpdir=_tmpdir)
    res = run_bass_kernel_spmd(nc, in_maps, list(range(NCORES)), **kw)

    jj = np.arange(B, dtype=np.float32)
    junk_col = (B - jj)[None, :]          # po[o, j] junk = 128 - j
    i_map, o_map = c["i_map"], c["o_map"]
    o_b = np.empty((B, OUT), np.float32)
    for cr in range(NCORES):
        r = res.results[cr]
        po = r["po"] - junk_col                       # [64, 128] colsums
        ob_c = po.T.copy()                            # [j, o_local]
        rows = r["rs"] - (i_map + 1)                  # rowsums minus junk
        np.add.at(ob_c, (i_map.ravel(), o_map.ravel()), rows.ravel())
        o_b[:, OC * cr:OC * (cr + 1)] = ob_c
    out = np.concatenate([x, o_b], axis=1)
    if _trace:
        return out, res
    return out
